# revision 5
# baseline (speedup 1.0000x reference)
"""Causal multi-head self-attention on 8 TRN2 NeuronCores.

Sharding: core = (batch b, head-group g): 4 batches x 2 groups of 8 heads.
Host pre-transposes all operands so every TensorE matmul contracts over the
partition dim with zero on-device transposes:

  phase 1a: qk^T[n, i]  = sum_k Wqk[n, k] xT[k, i]      (lhsT=WqkT blk, rhs=xT)
  phase 1b: v[j, n]     = sum_k xT[k, j] WvT[k, n]      (lhsT=xT blk,   rhs=WvT)
  phase 2 (per head, per 512-wide i-chunk, per 128-deep j-block):
            S^T[j, i]   = sum_d kT[d, j] qT[d, i]       (lhsT=kT blk,   rhs=qT)
            A^T         = exp(S^T / 8) * causal_mask    (ACT + DVE)
            Yaug^T[n,i] = sum_j v_aug[j, n] A^T[j, i]   (lhsT=v_aug,    rhs=A^T)
              where v_aug has a ones column: row 64 of Yaug^T = softmax denom l
            y^T         = Yaug^T[0:64] * (1/l)          (recip + partition bcast)
  phase 3:  out[i, o]   = sum_n yT[n, i] WpT[n, o]      (lhsT=yT blk,   rhs=WpT)

All matmul operands are float32r (TF32-like, 1 cycle/row at N>=256, ~1.5e-4
matmul rel err); PSUM accumulation is fp32.  Softmax skips max-subtraction
(scores are O(+-10), exp is safe in fp32) so the denominator comes free from
the ones-column trick.  The two per-batch head-group partials are summed on
the host at gather time.
"""

import numpy as np

import concourse.mybir as mybir
import concourse.tile as tile
from concourse import bacc
from concourse.bass_utils import run_bass_kernel_spmd

F32 = mybir.dt.float32
F32R = mybir.dt.float32r
Exp = mybir.ActivationFunctionType.Exp

B, C, H = 4, 1024, 16
HPC = 8            # heads per core
HD = 64            # head dim
GQ = HPC * HD      # 512 columns per head group
P = 128
KB = C // P        # 8 k-blocks
SCALE = 0.125      # 1/sqrt(HD)


def build(T=2048, ps1_bufs=3, psS_bufs=3, psY_bufs=2, at_bufs=3):
    nT = T // P      # j-blocks
    nI = T // 512    # i-chunks
    nc = bacc.Bacc("TRN2", target_bir_lowering=False, debug=False, num_devices=8)

    xT = nc.dram_tensor("xT", [C, T], F32R, kind="ExternalInput").ap()
    wqkT = nc.dram_tensor("wqkT", [C, 2 * GQ], F32R, kind="ExternalInput").ap()
    wvT = nc.dram_tensor("wvT", [C, GQ], F32R, kind="ExternalInput").ap()
    wpT = nc.dram_tensor("wpT", [GQ, C], F32R, kind="ExternalInput").ap()
    maskT = nc.dram_tensor("maskT", [P, 2 * P], F32R, kind="ExternalInput").ap()
    onesT = nc.dram_tensor("onesT", [P, (T // P) * HPC], F32R, kind="ExternalInput").ap()
    out = nc.dram_tensor("out", [T, C], F32, kind="ExternalOutput").ap()

    with tile.TileContext(nc) as tc:
        with tc.tile_pool(name="persist", bufs=1) as pe, \
             tc.tile_pool(name="ps1", bufs=ps1_bufs, space="PSUM") as ps1, \
             tc.tile_pool(name="psS", bufs=psS_bufs, space="PSUM") as psS, \
             tc.tile_pool(name="psY", bufs=psY_bufs, space="PSUM") as psY:

            qk_sb = pe.tile([P, 8 * T], F32R, tag="qk")      # n-blocks 0-3 q, 4-7 k
            v_sb = pe.tile([P, nT * HPC * (HD + 1)], F32R, tag="v")
            mask_sb = pe.tile([P, 2 * P], F32R, tag="mask")
            nc.sync.dma_start(mask_sb[:], maskT)
            nc.sync.dma_start(
                v_sb[:].rearrange("p (j h w) -> p j h w", j=nT, h=HPC)[:, :, :, HD:HD + 1],
                onesT.rearrange("p (j h) -> p j h", j=nT)[:, :, :, None])

            with tc.tile_pool(name="ph1", bufs=1) as p1:
                x_sb = p1.tile([P, KB * T], F32R, tag="x")
                wv_sb = p1.tile([P, KB * GQ], F32R, tag="wv")
                for kb in range(KB):
                    nc.sync.dma_start(x_sb[:, kb * T:(kb + 1) * T], xT[kb * P:(kb + 1) * P, :])
                    nc.sync.dma_start(wv_sb[:, kb * GQ:(kb + 1) * GQ], wvT[kb * P:(kb + 1) * P, :])

                # ---- phase 1b: v = x @ Wv^T, with ones column appended per head ----
                for jb in range(nT):
                    pt = ps1.tile([P, GQ], F32, tag="ps1")
                    for kb in range(KB):
                        nc.tensor.matmul(
                            pt[:],
                            x_sb[:, kb * T + jb * P: kb * T + (jb + 1) * P],
                            wv_sb[:, kb * GQ:(kb + 1) * GQ],
                            start=(kb == 0), stop=(kb == KB - 1))
                    vv = v_sb[:, jb * HPC * (HD + 1):(jb + 1) * HPC * (HD + 1)] \
                        .rearrange("p (h w) -> p h w", h=HPC)
                    nc.vector.tensor_copy(vv[:, :, 0:HD], pt[:].rearrange("p (h w) -> p h w", h=HPC))

                # ---- phase 1a: qk^T = Wqk @ x (n-blocks of 128 rows) ----
                for half in (0, 1):
                    with tc.tile_pool(name=f"wqk{half}", bufs=1) as pw:
                        w_sb = pw.tile([P, KB * GQ], F32R, tag=f"w{half}")
                        for kb in range(KB):
                            nc.sync.dma_start(
                                w_sb[:, kb * GQ:(kb + 1) * GQ],
                                wqkT[kb * P:(kb + 1) * P, half * GQ:(half + 1) * GQ])
                        for nb in range(4):
                            for mc in range(nI):
                                pt = ps1.tile([P, 512], F32, tag="ps1")
                                for kb in range(KB):
                                    nc.tensor.matmul(
                                        pt[:],
                                        w_sb[:, kb * GQ + nb * P: kb * GQ + (nb + 1) * P],
                                        x_sb[:, kb * T + mc * 512: kb * T + (mc + 1) * 512],
                                        start=(kb == 0), stop=(kb == KB - 1))
                                nc.vector.tensor_copy(
                                    qk_sb[:, (4 * half + nb) * T + mc * 512:
                                          (4 * half + nb) * T + (mc + 1) * 512], pt[:])

            # ---- phases 2+3 ----
            with tc.tile_pool(name="p23", bufs=1) as p23, \
                 tc.tile_pool(name="wrk", bufs=at_bufs) as wrk, \
                 tc.tile_pool(name="fin", bufs=2) as fin:
                yt_sb = p23.tile([P, 4 * T], F32R, tag="yt")
                wp_sb = p23.tile([P, 4 * C], F32R, tag="wp")
                phase23(nc, tc, T, nT, nI, out, qk_sb, v_sb, mask_sb,
                        yt_sb, wp_sb, wpT, wrk, fin, ps1, psS, psY)
    return nc


def phase23(nc, tc, T, nT, nI, out, qk_sb, v_sb, mask_sb, yt_sb, wp_sb, wpT,
            wrk, fin, ps1, psS, psY):
            for kb in range(4):
                nc.sync.dma_start(wp_sb[:, kb * C:(kb + 1) * C], wpT[kb * P:(kb + 1) * P, :])

            for h in range(HPC):
                po = (h % 2) * HD                 # partition offset of this head's d rows
                qc = (h // 2) * T                 # col base of q n-block
                kc = (4 + h // 2) * T             # col base of k n-block
                vc = h * (HD + 1)                 # col base inside v_aug j-block
                for ci in range(nI):
                    jmax = 4 * ci + 4
                    py = psY.tile([HD + 1, 512], F32, tag="psY")
                    for jb in range(jmax):
                        p_ = jb - 4 * ci
                        a = 0 if p_ < 1 else (256 if p_ == 3 else 128 * p_)
                        ps_ = psS.tile([P, 512], F32, tag="psS")
                        nc.tensor.matmul(
                            ps_[:, a:512],
                            qk_sb[po:po + HD, kc + jb * P: kc + (jb + 1) * P],
                            qk_sb[po:po + HD, qc + ci * 512 + a: qc + (ci + 1) * 512],
                            start=True, stop=True)
                        at = wrk.tile([P, 512], F32R, tag="at")
                        if p_ == 3:
                            nc.scalar.activation(at[:, 256:512], ps_[:, 256:512], Exp, scale=SCALE)
                            nc.vector.tensor_mul(at[:, 256:512], at[:, 256:512], mask_sb[:])
                        elif p_ >= 0:
                            nc.scalar.activation(at[:, a:512], ps_[:, a:512], Exp, scale=SCALE)
                            nc.vector.tensor_mul(at[:, a:a + P], at[:, a:a + P], mask_sb[:, P:2 * P])
                        else:
                            nc.scalar.activation(at[:, :], ps_[:, :], Exp, scale=SCALE)
                        nc.tensor.matmul(
                            py[:, a:512],
                            v_sb[:, jb * HPC * (HD + 1) + vc: jb * HPC * (HD + 1) + vc + HD + 1],
                            at[:, a:512],
                            start=(jb == 0), stop=(jb == jmax - 1))
                    rt = fin.tile([1, 512], F32, tag="rt")
                    nc.vector.reciprocal(rt[:], py[HD:HD + 1, :])
                    rb = fin.tile([HD, 512], F32, tag="rb")
                    nc.gpsimd.partition_broadcast(rb[:], rt[:])
                    nc.vector.tensor_mul(
                        yt_sb[po:po + HD, qc + ci * 512: qc + (ci + 1) * 512],
                        py[0:HD, :], rb[:])

            # ---- phase 3: out = y @ Wp^T (partial; host sums head-group pairs) ----
            for mb in range(nT):
                for oc in range(2):
                    po_ = ps1.tile([P, 512], F32, tag="ps1")
                    for nb in range(4):
                        nc.tensor.matmul(
                            po_[:],
                            yt_sb[:, nb * T + mb * P: nb * T + (mb + 1) * P],
                            wp_sb[:, nb * C + oc * 512: nb * C + (oc + 1) * 512],
                            start=(nb == 0), stop=(nb == 3))
                    ot = wrk.tile([P, 512], F32, tag="ot")
                    nc.vector.tensor_copy(ot[:], po_[:])
                    nc.sync.dma_start(out[mb * P:(mb + 1) * P, oc * 512:(oc + 1) * 512], ot[:])


_CACHE = {}


def get_nc(T=2048):
    if T not in _CACHE:
        nc = build(T)
        nc.compile()
        _CACHE[T] = nc
    return _CACHE[T]


def make_in_maps(x, W_attn, W_proj):
    Bx, T, Cx = x.shape
    Wq, Wk, Wv = W_attn[:Cx], W_attn[Cx:2 * Cx], W_attn[2 * Cx:]
    r = np.arange(P)
    tri = (r[:, None] <= r[None, :]).astype(np.float32)
    mask = np.concatenate([np.zeros((P, P), np.float32), tri], axis=1)
    ones = np.ones((P, (T // P) * HPC), np.float32)
    in_maps = []
    for core in range(8):
        b, g = divmod(core, 2)
        rows = slice(g * GQ, (g + 1) * GQ)
        in_maps.append({
            "xT": np.ascontiguousarray(x[b].T),
            "wqkT": np.ascontiguousarray(
                np.concatenate([Wq[rows], Wk[rows]], 0).T),
            "wvT": np.ascontiguousarray(Wv[rows].T),
            "wpT": np.ascontiguousarray(W_proj[:, rows].T),
            "maskT": mask,
            "onesT": ones,
        })
    return in_maps


def kernel(x, W_attn, W_proj):
    x = np.asarray(x, dtype=np.float32)
    W_attn = np.asarray(W_attn, dtype=np.float32)
    W_proj = np.asarray(W_proj, dtype=np.float32)
    Bx, T, Cx = x.shape
    assert (Bx, Cx) == (B, C) and W_attn.shape == (3 * C, C) and W_proj.shape == (C, C)
    nc = get_nc(T)
    res = run_bass_kernel_spmd(nc, make_in_maps(x, W_attn, W_proj), list(range(8)))
    out = np.empty((Bx, T, Cx), np.float32)
    for b in range(Bx):
        out[b] = res.results[2 * b]["out"] + res.results[2 * b + 1]["out"]
    return out


if __name__ == "__main__":
    rng = np.random.default_rng(0)
    x = rng.standard_normal((B, 2048, C), dtype=np.float32)
    W_attn = rng.standard_normal((3 * C, C), dtype=np.float32) * (1.0 / np.sqrt(C))
    W_proj = rng.standard_normal((C, C), dtype=np.float32) * (1.0 / np.sqrt(C))
    out = kernel(x, W_attn, W_proj)
    print("out", out.shape, out.dtype, np.abs(out).max())


# revision 6
# speedup vs baseline: 1.9007x; 1.9007x over previous
"""Causal multi-head self-attention on 8 TRN2 NeuronCores.

Sharding: core = (batch b, head-group g): 4 batches x 2 groups of 8 heads.
Host pre-transposes all operands so every TensorE matmul contracts over the
partition dim with zero on-device transposes:

  phase 1a: qk^T[n, i]  = sum_k Wqk[n, k] xT[k, i]      (lhsT=WqkT blk, rhs=xT)
  phase 1b: v[j, n]     = sum_k xT[k, j] WvT[k, n]      (lhsT=xT blk,   rhs=WvT)
  phase 2 (per head, per 512-wide i-chunk, per 128-deep j-block):
            S^T[j, i]   = sum_d kT[d, j] qT[d, i]       (lhsT=kT blk,   rhs=qT)
            A^T         = exp(S^T / 8) * causal_mask    (ACT + DVE)
            Yaug^T[n,i] = sum_j v_aug[j, n] A^T[j, i]   (lhsT=v_aug,    rhs=A^T)
              where v_aug has a ones column: row 64 of Yaug^T = softmax denom l
            y^T         = Yaug^T[0:64] * (1/l)          (recip + partition bcast)
  phase 3:  out[i, o]   = sum_n yT[n, i] WpT[n, o]      (lhsT=yT blk,   rhs=WpT)

All matmul operands are float32r (TF32-like, 1 cycle/row at N>=256, ~1.5e-4
matmul rel err); PSUM accumulation is fp32.  Softmax skips max-subtraction
(scores are O(+-10), exp is safe in fp32) so the denominator comes free from
the ones-column trick.  The two per-batch head-group partials are summed on
the host at gather time.
"""

import numpy as np

import concourse.mybir as mybir
import concourse.tile as tile
from concourse import bacc
from concourse.bass_utils import run_bass_kernel_spmd

F32 = mybir.dt.float32
F32R = mybir.dt.float32r
Exp = mybir.ActivationFunctionType.Exp

B, C, H = 4, 1024, 16
HPC = 8            # heads per core
HD = 64            # head dim
GQ = HPC * HD      # 512 columns per head group
P = 128
KB = C // P        # 8 k-blocks
SCALE = 0.125      # 1/sqrt(HD)


def build(T=2048, ps1_bufs=3, psS_bufs=3, psY_bufs=2, at_bufs=3, dup=1):
    nT = T // P      # j-blocks
    nI = T // 512    # i-chunks
    nc = bacc.Bacc("TRN2", target_bir_lowering=False, debug=False, num_devices=8)

    xT = nc.dram_tensor("xT", [C, T], F32R, kind="ExternalInput").ap()
    wqkT = nc.dram_tensor("wqkT", [C, 2 * GQ], F32R, kind="ExternalInput").ap()
    wvT = nc.dram_tensor("wvT", [C, GQ], F32R, kind="ExternalInput").ap()
    wpT = nc.dram_tensor("wpT", [GQ, C], F32R, kind="ExternalInput").ap()
    maskT = nc.dram_tensor("maskT", [P, 2 * P], F32R, kind="ExternalInput").ap()
    onesT = nc.dram_tensor("onesT", [P, (T // P) * HPC], F32R, kind="ExternalInput").ap()
    out = nc.dram_tensor("out", [T, C], F32, kind="ExternalOutput").ap()

    with tile.TileContext(nc) as tc:
      for _rep in range(dup):
        with tc.tile_pool(name="persist", bufs=1) as pe, \
             tc.tile_pool(name="ps1", bufs=ps1_bufs, space="PSUM") as ps1, \
             tc.tile_pool(name="psS", bufs=psS_bufs, space="PSUM") as psS, \
             tc.tile_pool(name="psY", bufs=psY_bufs, space="PSUM") as psY:

            qk_sb = pe.tile([P, 8 * T], F32R, tag="qk")      # n-blocks 0-3 q, 4-7 k
            v_sb = pe.tile([P, nT * HPC * (HD + 1)], F32R, tag="v")
            mask_sb = pe.tile([P, 2 * P], F32R, tag="mask")
            nc.sync.dma_start(mask_sb[:], maskT)
            nc.sync.dma_start(
                v_sb[:].rearrange("p (j h w) -> p j h w", j=nT, h=HPC)[:, :, :, HD:HD + 1],
                onesT.rearrange("p (j h) -> p j h", j=nT)[:, :, :, None])

            with tc.tile_pool(name="ph1", bufs=1) as p1:
                x_sb = p1.tile([P, KB * T], F32R, tag="x")
                wv_sb = p1.tile([P, KB * GQ], F32R, tag="wv")
                for kb in range(KB):
                    nc.sync.dma_start(x_sb[:, kb * T:(kb + 1) * T], xT[kb * P:(kb + 1) * P, :])
                    nc.sync.dma_start(wv_sb[:, kb * GQ:(kb + 1) * GQ], wvT[kb * P:(kb + 1) * P, :])

                # ---- phase 1b: v = x @ Wv^T, with ones column appended per head ----
                for jb in range(nT):
                    pt = ps1.tile([P, GQ], F32, tag="ps1")
                    for kb in range(KB):
                        nc.tensor.matmul(
                            pt[:],
                            x_sb[:, kb * T + jb * P: kb * T + (jb + 1) * P],
                            wv_sb[:, kb * GQ:(kb + 1) * GQ],
                            start=(kb == 0), stop=(kb == KB - 1))
                    vv = v_sb[:, jb * HPC * (HD + 1):(jb + 1) * HPC * (HD + 1)] \
                        .rearrange("p (h w) -> p h w", h=HPC)
                    nc.vector.tensor_copy(vv[:, :, 0:HD], pt[:].rearrange("p (h w) -> p h w", h=HPC))

                # ---- phase 1a: qk^T = Wqk @ x (n-blocks of 128 rows) ----
                for half in (0, 1):
                    with tc.tile_pool(name=f"wqk{half}", bufs=1) as pw:
                        w_sb = pw.tile([P, KB * GQ], F32R, tag=f"w{half}")
                        for kb in range(KB):
                            nc.sync.dma_start(
                                w_sb[:, kb * GQ:(kb + 1) * GQ],
                                wqkT[kb * P:(kb + 1) * P, half * GQ:(half + 1) * GQ])
                        for nb in range(4):
                            for mc in range(nI):
                                pt = ps1.tile([P, 512], F32, tag="ps1")
                                for kb in range(KB):
                                    nc.tensor.matmul(
                                        pt[:],
                                        w_sb[:, kb * GQ + nb * P: kb * GQ + (nb + 1) * P],
                                        x_sb[:, kb * T + mc * 512: kb * T + (mc + 1) * 512],
                                        start=(kb == 0), stop=(kb == KB - 1))
                                nc.vector.tensor_copy(
                                    qk_sb[:, (4 * half + nb) * T + mc * 512:
                                          (4 * half + nb) * T + (mc + 1) * 512], pt[:])

            # ---- phases 2+3 ----
            with tc.tile_pool(name="p23", bufs=1) as p23, \
                 tc.tile_pool(name="wrk", bufs=at_bufs) as wrk, \
                 tc.tile_pool(name="fin", bufs=2) as fin:
                yt_sb = p23.tile([P, 4 * T], F32R, tag="yt")
                wp_sb = p23.tile([P, 4 * C], F32R, tag="wp")
                phase23(nc, tc, T, nT, nI, out, qk_sb, v_sb, mask_sb,
                        yt_sb, wp_sb, wpT, wrk, fin, ps1, psS, psY)
    return nc


def phase23(nc, tc, T, nT, nI, out, qk_sb, v_sb, mask_sb, yt_sb, wp_sb, wpT,
            wrk, fin, ps1, psS, psY):
            for kb in range(4):
                nc.sync.dma_start(wp_sb[:, kb * C:(kb + 1) * C], wpT[kb * P:(kb + 1) * P, :])

            for h in range(HPC):
                po = (h % 2) * HD                 # partition offset of this head's d rows
                qc = (h // 2) * T                 # col base of q n-block
                kc = (4 + h // 2) * T             # col base of k n-block
                vc = h * (HD + 1)                 # col base inside v_aug j-block
                for ci in range(nI):
                    jmax = 4 * ci + 4
                    py = psY.tile([HD + 1, 512], F32, tag="psY")
                    for jb in range(jmax):
                        p_ = jb - 4 * ci
                        a = 0 if p_ < 1 else (256 if p_ == 3 else 128 * p_)
                        ps_ = psS.tile([P, 512], F32, tag="psS")
                        nc.tensor.matmul(
                            ps_[:, a:512],
                            qk_sb[po:po + HD, kc + jb * P: kc + (jb + 1) * P],
                            qk_sb[po:po + HD, qc + ci * 512 + a: qc + (ci + 1) * 512],
                            start=True, stop=True)
                        at = wrk.tile([P, 512], F32R, tag="at")
                        if p_ == 3:
                            nc.scalar.activation(at[:, 256:512], ps_[:, 256:512], Exp, scale=SCALE)
                            nc.vector.tensor_mul(at[:, 256:512], at[:, 256:512], mask_sb[:])
                        elif p_ >= 0:
                            nc.scalar.activation(at[:, a:512], ps_[:, a:512], Exp, scale=SCALE)
                            nc.vector.tensor_mul(at[:, a:a + P], at[:, a:a + P], mask_sb[:, P:2 * P])
                        else:
                            nc.scalar.activation(at[:, :], ps_[:, :], Exp, scale=SCALE)
                        nc.tensor.matmul(
                            py[:, a:512],
                            v_sb[:, jb * HPC * (HD + 1) + vc: jb * HPC * (HD + 1) + vc + HD + 1],
                            at[:, a:512],
                            start=(jb == 0), stop=(jb == jmax - 1))
                    rt = fin.tile([1, 512], F32, tag="rt")
                    nc.vector.reciprocal(rt[:], py[HD:HD + 1, :])
                    rb = fin.tile([HD, 512], F32, tag="rb")
                    nc.gpsimd.partition_broadcast(rb[:], rt[:])
                    nc.vector.tensor_mul(
                        yt_sb[po:po + HD, qc + ci * 512: qc + (ci + 1) * 512],
                        py[0:HD, :], rb[:])

            # ---- phase 3: out = y @ Wp^T (partial; host sums head-group pairs) ----
            for mb in range(nT):
                for oc in range(2):
                    po_ = ps1.tile([P, 512], F32, tag="ps1")
                    for nb in range(4):
                        nc.tensor.matmul(
                            po_[:],
                            yt_sb[:, nb * T + mb * P: nb * T + (mb + 1) * P],
                            wp_sb[:, nb * C + oc * 512: nb * C + (oc + 1) * 512],
                            start=(nb == 0), stop=(nb == 3))
                    ot = wrk.tile([P, 512], F32, tag="ot")
                    nc.vector.tensor_copy(ot[:], po_[:])
                    nc.sync.dma_start(out[mb * P:(mb + 1) * P, oc * 512:(oc + 1) * 512], ot[:])


_CACHE = {}


def get_nc(T=2048):
    if T not in _CACHE:
        nc = build(T)
        nc.compile()
        _CACHE[T] = nc
    return _CACHE[T]


def make_in_maps(x, W_attn, W_proj):
    Bx, T, Cx = x.shape
    Wq, Wk, Wv = W_attn[:Cx], W_attn[Cx:2 * Cx], W_attn[2 * Cx:]
    r = np.arange(P)
    tri = (r[:, None] <= r[None, :]).astype(np.float32)
    mask = np.concatenate([np.zeros((P, P), np.float32), tri], axis=1)
    ones = np.ones((P, (T // P) * HPC), np.float32)
    in_maps = []
    for core in range(8):
        b, g = divmod(core, 2)
        rows = slice(g * GQ, (g + 1) * GQ)
        in_maps.append({
            "xT": np.ascontiguousarray(x[b].T),
            "wqkT": np.ascontiguousarray(
                np.concatenate([Wq[rows], Wk[rows]], 0).T),
            "wvT": np.ascontiguousarray(Wv[rows].T),
            "wpT": np.ascontiguousarray(W_proj[:, rows].T),
            "maskT": mask,
            "onesT": ones,
        })
    return in_maps


def kernel(x, W_attn, W_proj):
    x = np.asarray(x, dtype=np.float32)
    W_attn = np.asarray(W_attn, dtype=np.float32)
    W_proj = np.asarray(W_proj, dtype=np.float32)
    Bx, T, Cx = x.shape
    assert (Bx, Cx) == (B, C) and W_attn.shape == (3 * C, C) and W_proj.shape == (C, C)
    nc = get_nc(T)
    res = run_bass_kernel_spmd(nc, make_in_maps(x, W_attn, W_proj), list(range(8)))
    out = np.empty((Bx, T, Cx), np.float32)
    for b in range(Bx):
        out[b] = res.results[2 * b]["out"] + res.results[2 * b + 1]["out"]
    return out


if __name__ == "__main__":
    rng = np.random.default_rng(0)
    x = rng.standard_normal((B, 2048, C), dtype=np.float32)
    W_attn = rng.standard_normal((3 * C, C), dtype=np.float32) * (1.0 / np.sqrt(C))
    W_proj = rng.standard_normal((C, C), dtype=np.float32) * (1.0 / np.sqrt(C))
    out = kernel(x, W_attn, W_proj)
    print("out", out.shape, out.dtype, np.abs(out).max())
